# revision 4
# baseline (speedup 1.0000x reference)
"""GCNConv Trainium2 kernel, 8-core SPMD.

Math: out = D^-1/2 A D^-1/2 (x W^T + b), A = adjacency (+self loops,
duplicate edges collapse to 1).

Reformulated aggregate-first so no cross-core communication is needed:
    s    = deg^-1/2                       (host, from dedup'd A)
    xt   = s ⊙ x                          (host, fp16)
    agg  = A @ xt                         (device matmul 1, row-sharded)
    aggs = A @ s                          (host matvec, feeds bias term)
    out  = s ⊙ ([agg, aggs] @ [W^T; b])   (device matmul 2 + fused scale)

Device per core c (rows r = c*1024 .. c*1024+1024):
  matmul 1: aggT[f, r] = sum_j xt[j, f] * AT[j, r]   (lhsT=xt tiles, rhs=AT
            slice tiles, fp16 operands, fp32 PSUM accumulation; A entries
            are 0/1 -> exact in fp16)
  matmul 2: out[r, o] = sum_f aggT[f, r] * Wt[f, o] + aggs[r] * b[o]
            then scaled by s[r] on PSUM->SBUF eviction (ACT Copy w/ scale).

Full inputs in, full outputs out; sharding is internal (each core gets its
own AT slice / aggs slice / s slice; xt, Wt, b broadcast).
"""

import functools
import numpy as np

N = 8192
D = 512
NCORES = 8
ROWS = N // NCORES          # 1024 output rows per core
P = 128
KT = N // P                 # 64 contraction tiles
FT = D // P                 # 4 feature tiles
NH = ROWS // D              # 2 row halves of 512 per core
MT = ROWS // P              # 8 output row chunks per core

_HALF = "float16"           # np dtype name for low-precision operands


def _kernel_body(tc, aps):
    import concourse.mybir as mybir

    nc = tc.nc
    at, xt, wt, brow, aggs, sc, out = (
        aps["at"], aps["xt"], aps["wt"], aps["brow"], aps["aggs"],
        aps["sc"], aps["out"],
    )
    half = mybir.dt.float16 if _HALF == "float16" else mybir.dt.bfloat16
    f32 = mybir.dt.float32

    with (
        tc.tile_pool(name="xt_pool", bufs=KT) as xt_pool,
        tc.tile_pool(name="at_pool", bufs=6) as at_pool,
        tc.tile_pool(name="psum", bufs=1, space="PSUM") as psum_pool,
        tc.tile_pool(name="aggT_pool", bufs=NH * FT) as aggT_pool,
        tc.tile_pool(name="out_pool", bufs=3) as out_pool,
        tc.tile_pool(name="const", bufs=1) as const,
    ):
        # constants / broadcast operands
        wt_sb = []
        for i in range(FT):
            w_t = const.tile([P, D], half, tag="wt", bufs=FT, name=f"wt{i}")
            nc.sync.dma_start(out=w_t[:], in_=wt[i * P:(i + 1) * P, :])
            wt_sb.append(w_t)
        b_sb = const.tile([1, D], half, tag="b", name="b_sb")
        nc.sync.dma_start(out=b_sb[:], in_=brow[:])
        aggs_sb = const.tile([1, ROWS], half, tag="aggs", name="aggs_sb")
        nc.sync.dma_start(out=aggs_sb[:], in_=aggs[:])
        s_sb = const.tile([P, MT], f32, tag="s", name="s_sb")
        nc.sync.dma_start(out=s_sb[:], in_=sc[:])

        # xt resident in SBUF: 64 tiles [128, 512] fp16
        xt_sb = []
        for k in range(KT):
            x_t = xt_pool.tile([P, D], half, tag="xt", name=f"xt{k}")
            nc.sync.dma_start(out=x_t[:], in_=xt[k * P:(k + 1) * P, :])
            xt_sb.append(x_t)

        # ---- matmul 1: aggregation  aggT[n][f] += xt[k][f].T @ at[k][n] ----
        psum = []
        for i in range(NH * FT):
            ps = psum_pool.tile([P, D], f32, tag=f"ps{i}", name=f"ps{i}")
            psum.append(ps)
        for k in range(KT):
            at_t = at_pool.tile([P, ROWS], half, tag="at", name=f"at{k}")
            nc.sync.dma_start(out=at_t[:], in_=at[k * P:(k + 1) * P, :])
            for f in range(FT):
                lhsT = xt_sb[k][:, f * P:(f + 1) * P]
                for n in range(NH):
                    nc.tensor.matmul(
                        psum[n * FT + f][:],
                        lhsT,
                        at_t[:, n * D:(n + 1) * D],
                        start=(k == 0),
                        stop=(k == KT - 1),
                    )

        # evict (fp32 -> fp16 cast)
        aggT = []
        for i in range(NH * FT):
            agg_t = aggT_pool.tile([P, D], half, tag="aggT", name=f"aggT{i}")
            nc.vector.tensor_copy(agg_t[:], psum[i][:])
            aggT.append(agg_t)

        # ---- matmul 2 + fused s-scale on eviction ----
        for m in range(MT):
            n, off = m // FT, (m % FT) * P
            # reuse the aggregation psum banks (same tag -> same slots)
            ps2 = psum_pool.tile([P, D], f32, tag=f"ps{m % 2}",
                                 name=f"ps2_{m}")
            for kf in range(FT):
                nc.tensor.matmul(
                    ps2[:],
                    aggT[n * FT + kf][:, off:off + P],
                    wt_sb[kf][:],
                    start=(kf == 0),
                    stop=False,
                )
            nc.tensor.matmul(
                ps2[:],
                aggs_sb[:, m * P:(m + 1) * P],
                b_sb[:],
                start=False,
                stop=True,
            )
            o_t = out_pool.tile([P, D], f32, tag="o", name=f"o{m}")
            nc.scalar.activation(
                o_t[:], ps2[:], mybir.ActivationFunctionType.Copy,
                scale=s_sb[:, m:m + 1],
            )
            nc.sync.dma_start(out=out[m * P:(m + 1) * P, :], in_=o_t[:])


@functools.lru_cache(maxsize=4)
def _build(repeat=1):
    import concourse.bacc as bacc
    import concourse.mybir as mybir
    import concourse.tile as tile

    half = mybir.dt.float16 if _HALF == "float16" else mybir.dt.bfloat16
    nc = bacc.Bacc("TRN2", target_bir_lowering=False, debug=False,
                   num_devices=NCORES)
    aps = {
        "at": nc.dram_tensor("at", [N, ROWS], half, kind="ExternalInput").ap(),
        "xt": nc.dram_tensor("xt", [N, D], half, kind="ExternalInput").ap(),
        "wt": nc.dram_tensor("wt", [D, D], half, kind="ExternalInput").ap(),
        "brow": nc.dram_tensor("brow", [1, D], half, kind="ExternalInput").ap(),
        "aggs": nc.dram_tensor("aggs", [1, ROWS], half,
                               kind="ExternalInput").ap(),
        "sc": nc.dram_tensor("sc", [P, MT], mybir.dt.float32,
                             kind="ExternalInput").ap(),
        "out": nc.dram_tensor("out", [ROWS, D], mybir.dt.float32,
                              kind="ExternalOutput").ap(),
    }
    with tile.TileContext(nc) as tc:
        for _ in range(repeat):
            _kernel_body(tc, aps)
    nc.compile()
    return nc


def _prep(x, edge_index, W, b):
    """Host-side index scatter + scaling; returns per-core input maps."""
    half = np.dtype(_HALF)
    ei = np.asarray(edge_index)
    # AT[j, r] = A[r, j]; duplicates collapse via assignment, + self loops
    AT = np.zeros((N, N), dtype=np.uint8)
    AT[ei[1].astype(np.int64), ei[0].astype(np.int64)] = 1
    idx = np.arange(N)
    AT[idx, idx] = 1
    deg = AT.sum(axis=0, dtype=np.int64).astype(np.float64)  # A row sums
    s = (1.0 / np.sqrt(deg)).astype(np.float32)
    aggs = (AT.T.astype(np.float32) @ s).astype(half)        # A @ s
    xt = (s[:, None] * np.asarray(x)).astype(half)
    wt = np.ascontiguousarray(np.asarray(W).T).astype(half)
    brow = np.asarray(b).reshape(1, D).astype(half)

    in_maps = []
    for c in range(NCORES):
        rows = slice(c * ROWS, (c + 1) * ROWS)
        in_maps.append({
            "at": np.ascontiguousarray(AT[:, rows]).astype(half),
            "xt": xt,
            "wt": wt,
            "brow": brow,
            "aggs": np.ascontiguousarray(aggs[rows]).reshape(1, ROWS),
            # sc[p, m] = s[c*1024 + m*128 + p]
            "sc": np.ascontiguousarray(
                s[rows].reshape(MT, P).T).astype(np.float32),
        })
    return in_maps


def kernel(x, edge_index, W, b):
    from concourse import bass_utils

    nc = _build()
    in_maps = _prep(x, edge_index, W, b)
    res = bass_utils.run_bass_kernel_spmd(
        nc, in_maps, core_ids=list(range(NCORES)))
    return np.concatenate(
        [res.results[c]["out"] for c in range(NCORES)], axis=0)


# revision 24
# speedup vs baseline: 69.9967x; 69.9967x over previous
"""GCNConv Trainium2 kernel, 8-core SPMD.

Math: out = D^-1/2 A D^-1/2 (x W^T + b), A = adjacency (+self loops,
duplicate edges collapse to 1).

Reformulated aggregate-first so no cross-core communication is needed:
    s    = deg^-1/2                       (host, from dedup'd A)
    xt   = s ⊙ x                          (host, fp16)
    agg  = A @ xt                         (device matmul 1, row-sharded)
    aggs = A @ s                          (host matvec, feeds bias term)
    out  = s ⊙ ([agg, aggs] @ [W^T; b])   (device matmul 2 + fused scale)

Device per core c (rows r = c*1024 .. c*1024+1024):
  matmul 1: aggT[f, r] = sum_j xt[j, f] * AT[j, r]   (lhsT=xt tiles, rhs=AT
            slice tiles, fp16 operands, fp32 PSUM accumulation; A entries
            are 0/1 -> exact in fp16)
  matmul 2: out[r, o] = sum_f aggT[f, r] * Wt[f, o] + aggs[r] * b[o]
            then scaled by s[r] on PSUM->SBUF eviction (ACT Copy w/ scale).

Full inputs in, full outputs out; sharding is internal (each core gets its
own AT slice / aggs slice / s slice; xt, Wt, b broadcast).
"""

import functools
import numpy as np

N = 8192
D = 512
NCORES = 8
ROWS = N // NCORES          # 1024 output rows per core
P = 128
KT = N // P                 # 64 contraction tiles
FT = D // P                 # 4 feature tiles
NH = ROWS // D              # 2 row halves of 512 per core
MT = ROWS // P              # 8 output row chunks per core

_HALF = "float16"           # np dtype name for low-precision operands


def _kernel_body(tc, aps, bufs=8, const_after_k=None):
    import concourse.mybir as mybir

    nc = tc.nc
    at, xt, wt, brow, aggs, sc, out = (
        aps["at"], aps["xt"], aps["wt"], aps["brow"], aps["aggs"],
        aps["sc"], aps["out"],
    )
    half = mybir.dt.float16 if _HALF == "float16" else mybir.dt.bfloat16
    fp8 = mybir.dt.float8e4
    f32 = mybir.dt.float32

    with (
        tc.tile_pool(name="xt_pool", bufs=bufs) as xt_pool,
        tc.tile_pool(name="at_pool", bufs=bufs) as at_pool,
        tc.tile_pool(name="psum", bufs=1, space="PSUM") as psum_pool,
        tc.tile_pool(name="aggT_pool", bufs=NH * FT) as aggT_pool,
        tc.tile_pool(name="out_pool", bufs=3) as out_pool,
        tc.tile_pool(name="const", bufs=1) as const,
    ):
        wt_sb = []
        b_sb = aggs_sb = s_sb = None

        def emit_consts():
            nonlocal b_sb, aggs_sb, s_sb
            for i in range(FT):
                w_t = const.tile([P, D], half, tag="wt", bufs=FT,
                                 name=f"wt{i}")
                nc.sync.dma_start(out=w_t[:], in_=wt[i * P:(i + 1) * P, :])
                wt_sb.append(w_t)
            b_sb = const.tile([1, D], half, tag="b", name="b_sb")
            nc.sync.dma_start(out=b_sb[:], in_=brow[:])
            aggs_sb = const.tile([1, ROWS], half, tag="aggs", name="aggs_sb")
            nc.sync.dma_start(out=aggs_sb[:], in_=aggs[:])
            s_sb = const.tile([P, MT], f32, tag="s", name="s_sb")
            nc.sync.dma_start(out=s_sb[:], in_=sc[:])

        if const_after_k is None:
            emit_consts()

        # ---- matmul 1: aggregation  aggT[n][f] += xt[k][f].T @ at[k][n] ----
        # at is fp8e4 (adjacency entries are 0/1, exact)
        psum = []
        for i in range(NH * FT):
            ps = psum_pool.tile([P, D], f32, tag=f"ps{i}", name=f"ps{i}")
            psum.append(ps)
        for k in range(KT):
            x_t = xt_pool.tile([P, D], half, tag="xt", name=f"xt{k}")
            nc.sync.dma_start(out=x_t[:], in_=xt[k * P:(k + 1) * P, :])
            at_t = at_pool.tile([P, ROWS], fp8, tag="at", name=f"at{k}")
            nc.sync.dma_start(out=at_t[:], in_=at[k * P:(k + 1) * P, :])
            if const_after_k == k:
                emit_consts()
            for f in range(FT):
                lhsT = x_t[:, f * P:(f + 1) * P]
                for n in range(NH):
                    nc.tensor.matmul(
                        psum[n * FT + f][:], lhsT,
                        at_t[:, n * D:(n + 1) * D],
                        start=(k == 0), stop=(k == KT - 1),
                    )


        # evict (fp32 -> fp16 cast)
        aggT = []
        for i in range(NH * FT):
            agg_t = aggT_pool.tile([P, D], half, tag="aggT", name=f"aggT{i}")
            nc.vector.tensor_copy(agg_t[:], psum[i][:])
            aggT.append(agg_t)

        # ---- matmul 2 + fused s-scale on eviction ----
        for m in range(MT):
            n, off = m // FT, (m % FT) * P
            # reuse the aggregation psum banks (same tag -> same slots)
            ps2 = psum_pool.tile([P, D], f32, tag=f"ps{m % 2}",
                                 name=f"ps2_{m}")
            for kf in range(FT):
                nc.tensor.matmul(
                    ps2[:],
                    aggT[n * FT + kf][:, off:off + P],
                    wt_sb[kf][:],
                    start=(kf == 0),
                    stop=False,
                )
            nc.tensor.matmul(
                ps2[:],
                aggs_sb[:, m * P:(m + 1) * P],
                b_sb[:],
                start=False,
                stop=True,
            )
            o_t = out_pool.tile([P, D], f32, tag="o", name=f"o{m}")
            nc.scalar.activation(
                o_t[:], ps2[:], mybir.ActivationFunctionType.Copy,
                scale=s_sb[:, m:m + 1],
            )
            nc.sync.dma_start(out=out[m * P:(m + 1) * P, :], in_=o_t[:])


@functools.lru_cache(maxsize=8)
def _build(repeat=1, bufs=8, const_after_k=3):
    import concourse.bacc as bacc
    import concourse.mybir as mybir
    import concourse.tile as tile

    half = mybir.dt.float16 if _HALF == "float16" else mybir.dt.bfloat16
    nc = bacc.Bacc("TRN2", target_bir_lowering=False, debug=False,
                   num_devices=NCORES)
    aps = {
        "at": nc.dram_tensor("at", [N, ROWS], mybir.dt.float8e4,
                             kind="ExternalInput").ap(),
        "xt": nc.dram_tensor("xt", [N, D], half, kind="ExternalInput").ap(),
        "wt": nc.dram_tensor("wt", [D, D], half, kind="ExternalInput").ap(),
        "brow": nc.dram_tensor("brow", [1, D], half, kind="ExternalInput").ap(),
        "aggs": nc.dram_tensor("aggs", [1, ROWS], half,
                               kind="ExternalInput").ap(),
        "sc": nc.dram_tensor("sc", [P, MT], mybir.dt.float32,
                             kind="ExternalInput").ap(),
        "out": nc.dram_tensor("out", [ROWS, D], mybir.dt.float32,
                              kind="ExternalOutput").ap(),
    }
    with tile.TileContext(nc) as tc:
        for _ in range(repeat):
            _kernel_body(tc, aps, bufs=bufs, const_after_k=const_after_k)
    nc.compile()
    return nc


def _prep(x, edge_index, W, b):
    """Host-side index scatter + scaling; returns per-core input maps."""
    import ml_dtypes
    half = np.dtype(_HALF)
    fp8 = ml_dtypes.float8_e4m3
    ei = np.asarray(edge_index)
    # AT[j, r] = A[r, j]; duplicates collapse via assignment, + self loops
    AT = np.zeros((N, N), dtype=np.uint8)
    AT[ei[1].astype(np.int64), ei[0].astype(np.int64)] = 1
    idx = np.arange(N)
    AT[idx, idx] = 1
    deg = AT.sum(axis=0, dtype=np.int64).astype(np.float64)  # A row sums
    s = (1.0 / np.sqrt(deg)).astype(np.float32)
    aggs = (AT.T.astype(np.float32) @ s).astype(half)        # A @ s
    xt = (s[:, None] * np.asarray(x)).astype(half)
    wt = np.ascontiguousarray(np.asarray(W).T).astype(half)
    brow = np.asarray(b).reshape(1, D).astype(half)

    in_maps = []
    for c in range(NCORES):
        rows = slice(c * ROWS, (c + 1) * ROWS)
        in_maps.append({
            "at": np.ascontiguousarray(AT[:, rows]).astype(fp8),
            "xt": xt,
            "wt": wt,
            "brow": brow,
            "aggs": np.ascontiguousarray(aggs[rows]).reshape(1, ROWS),
            # sc[p, m] = s[c*1024 + m*128 + p]
            "sc": np.ascontiguousarray(
                s[rows].reshape(MT, P).T).astype(np.float32),
        })
    return in_maps


def kernel(x, edge_index, W, b):
    import time
    from concourse import bass_utils

    nc = _build()
    in_maps = _prep(x, edge_index, W, b)
    last = None
    for attempt in range(3):
        try:
            res = bass_utils.run_bass_kernel_spmd(
                nc, in_maps, core_ids=list(range(NCORES)))
            return np.concatenate(
                [res.results[c]["out"] for c in range(NCORES)], axis=0)
        except Exception as e:  # transient NRT device flakes recover on retry
            last = e
            time.sleep(5.0)
    raise last
